# revision 1
# baseline (speedup 1.0000x reference)
"""Trainium2 Bass/Tile kernel for per-patch multi-head attention.

Problem: x [B=4, P=4, N=1024, C=512]; per-patch Wq [P, C, C], Wkv [P, C, 2C];
shared Wproj [C, C], bproj [C]. 8 heads, hd=64.

Sharding: the 16 (b, p) pairs are fully independent; each of the 8 cores
processes 2 pairs (data/expert parallel, no collectives). Wproj/bproj are
replicated.

Per-core layout strategy (all matmul operands bf16, accum fp32):
  - xT [c, n] built via PE-transpose of DMA'd x tiles.
  - qT/kT [d, n] = Wq/Wk[c,d].T-contracted against xT (d head-major).
  - v [m, d] with a ones column appended per 64-wide head block, so the
    attention-times-v matmul also yields softmax denominators for free.
  - scoresT [m, n] per head via K=64 matmuls; heads are processed in pairs
    living at partition offsets 0/64 so the PE row-tiles them concurrently.
  - exp on the scalar engine straight out of PSUM in [128, 2048] slabs
    (one ACTIVATE per slab amortizes the ~352-cycle fixed cost).
  - out = expT.T @ [v|1] accumulated over m-chunks -> [n, 65] per head;
    reciprocal of col 64 normalizes via a per-partition tensor_scalar.
  - o [n, c] is PE-transposed to oT [c, n] and projected with Wproj; bias is
    added by a K=1 ones-row matmul seeding the PSUM accumulation.
"""

import numpy as np

import concourse.bass as bass
import concourse.bacc as bacc
import concourse.mybir as mybir
from concourse.masks import make_identity
from concourse.tile import TileContext

B, P, N, C = 4, 4, 1024, 512
HEADS = 8
HD = C // HEADS  # 64
NT = N // 128  # 8 n-tiles
CCH = C // 128  # 4 c-chunks
F32 = mybir.dt.float32
BF16 = mybir.dt.bfloat16

_CACHE = {}


def _build_kernel():
    nc = bacc.Bacc()
    x = nc.declare_dram_parameter("x", [2, N, C], F32, False)
    wq = nc.declare_dram_parameter("wq", [2, C, C], F32, False)
    wkv = nc.declare_dram_parameter("wkv", [2, C, 2 * C], F32, False)
    wproj = nc.declare_dram_parameter("wproj", [C, C], F32, False)
    bproj = nc.declare_dram_parameter("bproj", [1, C], F32, False)
    y = nc.declare_dram_parameter("y", [2, N, C], F32, True)

    with TileContext(nc) as tc:
        with (
            tc.tile_pool(name="consts", bufs=1) as consts,
            tc.tile_pool(name="wpool", bufs=2) as wpool,
            tc.tile_pool(name="xload", bufs=3) as xload,
            tc.tile_pool(name="bigp", bufs=2) as bigp,
            tc.tile_pool(name="opool", bufs=1) as opool,
            tc.tile_pool(name="expp", bufs=10) as expp,
            tc.tile_pool(name="smallp", bufs=8) as smallp,
            tc.tile_pool(name="ps_slab", bufs=1, space="PSUM") as ps_slab,
            tc.tile_pool(name="ps_av", bufs=2, space="PSUM") as ps_av,
            tc.tile_pool(name="ps_mm", bufs=1, space="PSUM") as ps_mm,
        ):
            ident32 = consts.tile([128, 128], F32)
            make_identity(nc, ident32)
            identbf = consts.tile([128, 128], BF16)
            make_identity(nc, identbf)
            ones_bf = consts.tile([1, 128], BF16)
            nc.vector.memset(ones_bf, 1.0)

            wproj_sb = []
            for ci in range(CCH):
                t32 = xload.tile([128, 512], F32, tag="wload", name="wload")
                nc.gpsimd.dma_start(out=t32, in_=wproj[ci * 128 : (ci + 1) * 128, :])
                tb = consts.tile([128, 512], BF16, tag=f"wproj{ci}", name=f"wproj{ci}")
                nc.vector.tensor_copy(tb, t32)
                wproj_sb.append(tb)
            bp32 = consts.tile([1, 512], F32)
            nc.gpsimd.dma_start(out=bp32, in_=bproj[:, :])
            bp_bf = consts.tile([1, 512], BF16)
            nc.vector.tensor_copy(bp_bf, bp32)

            for pr in range(2):
                # ---- per-patch weights, cast to bf16
                wq_sb, wk_sb, wv_sb = [], [], []
                for ci in range(CCH):
                    rows = slice(ci * 128, (ci + 1) * 128)
                    for lst, src, tag in (
                        (wq_sb, wq[pr, rows, :], f"wq{ci}"),
                        (wk_sb, wkv[pr, rows, 0:512], f"wk{ci}"),
                        (wv_sb, wkv[pr, rows, 512:1024], f"wv{ci}"),
                    ):
                        t32 = xload.tile([128, 512], F32, tag="wload", name="wload")
                        nc.gpsimd.dma_start(out=t32, in_=src)
                        tb = wpool.tile([128, 512], BF16, tag=tag, name=tag)
                        nc.vector.tensor_copy(tb, t32)
                        lst.append(tb)

                # ---- xT [c, n] via PE transpose
                xT = [bigp.tile([128, N], BF16, tag=f"xT{ci}", name=f"xT{ci}") for ci in range(CCH)]
                for nt in range(NT):
                    xt32 = xload.tile([128, 512], F32, tag="xload")
                    nc.gpsimd.dma_start(out=xt32, in_=x[pr, nt * 128 : (nt + 1) * 128, :])
                    for ci in range(CCH):
                        pst = ps_mm.tile([128, 128], F32, tag="tr")
                        nc.tensor.transpose(
                            pst, xt32[:, ci * 128 : (ci + 1) * 128], ident32
                        )
                        nc.vector.tensor_copy(
                            xT[ci][:, nt * 128 : (nt + 1) * 128], pst
                        )

                # ---- qT/kT [d, n] (d head-major: d-chunk di = heads 2di, 2di+1)
                qT = [bigp.tile([128, N], BF16, tag=f"qT{di}", name=f"qT{di}") for di in range(CCH)]
                kT = [bigp.tile([128, N], BF16, tag=f"kT{di}", name=f"kT{di}") for di in range(CCH)]
                for di in range(CCH):
                    dcols = slice(di * 128, (di + 1) * 128)
                    for nf in range(2):
                        ncols = slice(nf * 512, (nf + 1) * 512)
                        for dst, wsb in ((qT, wq_sb), (kT, wk_sb)):
                            ps = ps_mm.tile([128, 512], F32, tag="mm512")
                            for ci in range(CCH):
                                nc.tensor.matmul(
                                    ps,
                                    wsb[ci][:, dcols],
                                    xT[ci][:, ncols],
                                    start=(ci == 0),
                                    stop=(ci == CCH - 1),
                                )
                            nc.vector.tensor_copy(dst[di][:, ncols], ps)

                # ---- v [m, d] padded with a ones column per head block
                vpad = [bigp.tile([128, HEADS * 65], BF16, tag=f"v{mt}", name=f"v{mt}") for mt in range(NT)]
                for mt in range(NT):
                    ps = ps_mm.tile([128, 512], F32, tag="mm512")
                    for ci in range(CCH):
                        nc.tensor.matmul(
                            ps,
                            xT[ci][:, mt * 128 : (mt + 1) * 128],
                            wv_sb[ci],
                            start=(ci == 0),
                            stop=(ci == CCH - 1),
                        )
                    vv = vpad[mt].rearrange("p (h w) -> p h w", w=65)
                    nc.vector.memset(vv[:, :, 64:65], 1.0)
                    nc.vector.tensor_copy(
                        vv[:, :, 0:64], ps.rearrange("p (h w) -> p h w", w=64)
                    )

                # ---- attention, head pairs (2di, 2di+1) row-tiled on the PE
                o_sb = [opool.tile([128, C], BF16, tag=f"o{nt}", name=f"o{nt}") for nt in range(NT)]
                for di in range(CCH):
                    exps = []
                    for mt in range(NT):
                        slab = ps_slab.tile([128, 2048], F32, tag="slab")
                        for half in range(2):
                            prow = slice(half * 64, (half + 1) * 64)
                            for nf in range(2):
                                nc.tensor.matmul(
                                    slab[
                                        :,
                                        half * 1024
                                        + nf * 512 : half * 1024
                                        + (nf + 1) * 512,
                                    ],
                                    kT[di][prow, mt * 128 : (mt + 1) * 128],
                                    qT[di][prow, nf * 512 : (nf + 1) * 512],
                                    start=True,
                                    stop=True,
                                )
                        et = expp.tile([128, 2048], BF16, tag="exp")
                        nc.scalar.activation(
                            et, slab, mybir.ActivationFunctionType.Exp, scale=0.125
                        )
                        exps.append(et)
                    for nt in range(NT):
                        av = ps_av.tile([128, 130], F32, tag="av")
                        for half in range(2):
                            h = 2 * di + half
                            for mt in range(NT):
                                nc.tensor.matmul(
                                    av[:, half * 65 : (half + 1) * 65],
                                    exps[mt][
                                        :,
                                        half * 1024
                                        + nt * 128 : half * 1024
                                        + nt * 128
                                        + 128,
                                    ],
                                    vpad[mt][:, h * 65 : (h + 1) * 65],
                                    start=(mt == 0),
                                    stop=(mt == NT - 1),
                                )
                        for half in range(2):
                            h = 2 * di + half
                            rc = smallp.tile([128, 1], F32, tag="recip")
                            nc.vector.reciprocal(
                                rc, av[:, half * 65 + 64 : half * 65 + 65]
                            )
                            nc.vector.tensor_scalar_mul(
                                o_sb[nt][:, h * 64 : (h + 1) * 64],
                                av[:, half * 65 : half * 65 + 64],
                                rc,
                            )

                # ---- oT + proj + bias, stream out
                for nt in range(NT):
                    oTs = []
                    for ci in range(CCH):
                        pst = ps_mm.tile([128, 128], BF16, tag="tr")
                        nc.tensor.transpose(
                            pst, o_sb[nt][:, ci * 128 : (ci + 1) * 128], identbf
                        )
                        ot = smallp.tile([128, 128], BF16, tag="oT")
                        nc.vector.tensor_copy(ot, pst)
                        oTs.append(ot)
                    zps = ps_mm.tile([128, 512], F32, tag="mm512")
                    nc.tensor.matmul(
                        zps, ones_bf[0:1, :], bp_bf[0:1, :], start=True, stop=False
                    )
                    for ci in range(CCH):
                        nc.tensor.matmul(
                            zps,
                            oTs[ci],
                            wproj_sb[ci],
                            start=False,
                            stop=(ci == CCH - 1),
                        )
                    zsb = smallp.tile([128, 512], F32, tag="z")
                    nc.vector.tensor_copy(zsb, zps)
                    nc.gpsimd.dma_start(
                        out=y[pr, nt * 128 : (nt + 1) * 128, :], in_=zsb
                    )
    return nc


def _get_nc():
    if "nc" not in _CACHE:
        nc = _build_kernel()
        nc.compile()
        _CACHE["nc"] = nc
    return _CACHE["nc"]


def kernel(**inputs) -> np.ndarray:
    from concourse.bass_utils import run_bass_kernel_spmd

    x = np.ascontiguousarray(np.asarray(inputs["x"], dtype=np.float32))
    Wq = np.ascontiguousarray(np.asarray(inputs["Wq"], dtype=np.float32))
    Wkv = np.ascontiguousarray(np.asarray(inputs["Wkv"], dtype=np.float32))
    Wproj = np.ascontiguousarray(np.asarray(inputs["Wproj"], dtype=np.float32))
    bproj = np.ascontiguousarray(
        np.asarray(inputs["bproj"], dtype=np.float32).reshape(1, C)
    )

    nc = _get_nc()
    xr = x.reshape(B * P, N, C)
    in_maps = []
    for core in range(8):
        p0 = (2 * core) % P
        in_maps.append(
            {
                "x": np.ascontiguousarray(xr[2 * core : 2 * core + 2]),
                "wq": np.ascontiguousarray(Wq[p0 : p0 + 2]),
                "wkv": np.ascontiguousarray(Wkv[p0 : p0 + 2]),
                "wproj": Wproj,
                "bproj": bproj,
            }
        )
    res = run_bass_kernel_spmd(nc, in_maps, list(range(8))).results
    out = np.concatenate([r["y"] for r in res], axis=0).reshape(B, P, N, C)
    return out.astype(np.float32)



# revision 2
# speedup vs baseline: 1.4733x; 1.4733x over previous
"""Trainium2 Bass/Tile kernel for per-patch multi-head attention.

Problem: x [B=4, P=4, N=1024, C=512]; per-patch Wq [P, C, C], Wkv [P, C, 2C];
shared Wproj [C, C], bproj [C]. 8 heads, hd=64.

Sharding: the 16 (b, p) pairs are fully independent; each of the 8 cores
processes 2 pairs (data/expert parallel, no collectives). Wproj/bproj are
replicated.

Per-core pipeline (all matmul operands bf16, accum fp32):
  - x is cast to bf16 then PE-transposed (bf16 transposes are 4x cheaper than
    fp32); transposes land in bitcast views of the idle mm-pool PSUM bank.
  - qT/kT [d, n] computed lazily per head-pair di; scores for head pair di are
    row-tiled (two 64-row K slices of the PE run concurrently), one PSUM slab
    [128, 1024] per (mt, half), double buffered so the scalar engine's exp
    never blocks the next slab's matmuls.
  - exp on the scalar engine straight out of PSUM into SBUF bf16 tiles.
  - v [m, d] has a ones column per 64-wide head block, so attn @ [v|1] also
    yields softmax denominators; out [n, 65] per head accumulates over m in a
    single PSUM bank; reciprocal + per-partition tensor_scalar normalizes.
  - issue order interleaves scores(di) with AV(di-1) one slab group at a time
    so the tensor engine always has ready work while ACT drains exp slabs.
  - o [n, c] is PE-transposed (bf16) and projected with Wproj; bias is added
    by a K=1 ones-row matmul seeding the PSUM accumulation.
"""

import numpy as np

import concourse.bass as bass
import concourse.bacc as bacc
import concourse.mybir as mybir
from concourse.masks import make_identity
from concourse.tile import TileContext

B, P, N, C = 4, 4, 1024, 512
HEADS = 8
HD = C // HEADS  # 64
NT = N // 128  # 8 n-tiles
CCH = C // 128  # 4 c-chunks
F32 = mybir.dt.float32
BF16 = mybir.dt.bfloat16

_CACHE = {}


def _build_kernel():
    nc = bacc.Bacc()
    x = nc.declare_dram_parameter("x", [2, N, C], F32, False)
    wq = nc.declare_dram_parameter("wq", [2, C, C], F32, False)
    wkv = nc.declare_dram_parameter("wkv", [2, C, 2 * C], F32, False)
    wproj = nc.declare_dram_parameter("wproj", [C, C], F32, False)
    bproj = nc.declare_dram_parameter("bproj", [1, C], F32, False)
    y = nc.declare_dram_parameter("y", [2, N, C], F32, True)

    with TileContext(nc) as tc:
        with (
            tc.tile_pool(name="consts", bufs=1) as consts,
            tc.tile_pool(name="wpool", bufs=2) as wpool,
            tc.tile_pool(name="stage", bufs=3) as stage,
            tc.tile_pool(name="xTp", bufs=2) as xTp,
            tc.tile_pool(name="qkp", bufs=2) as qkp,
            tc.tile_pool(name="vp", bufs=2) as vp,
            tc.tile_pool(name="expp", bufs=2) as expp,
            tc.tile_pool(name="op", bufs=2) as op,
            tc.tile_pool(name="zp", bufs=3) as zp,
            tc.tile_pool(name="smallp", bufs=4) as smallp,
            tc.tile_pool(name="ps_slab", bufs=2, space="PSUM") as ps_slab,
            tc.tile_pool(name="ps_mm", bufs=2, space="PSUM") as ps_mm,
            tc.tile_pool(name="ps_av", bufs=2, space="PSUM") as ps_av,
        ):
            identbf = consts.tile([128, 128], BF16)
            make_identity(nc, identbf)
            ones_bf = consts.tile([1, 128], BF16)
            nc.vector.memset(ones_bf, 1.0)

            # shared proj weights + bias (replicated across pairs)
            wproj_sb = []
            for ci in range(CCH):
                t32 = stage.tile([128, 512], F32, tag="ws", name="ws")
                nc.sync.dma_start(out=t32, in_=wproj[ci * 128 : (ci + 1) * 128, :])
                tb = consts.tile([128, 512], BF16, tag=f"wproj{ci}", name=f"wproj{ci}")
                nc.vector.tensor_copy(tb, t32)
                wproj_sb.append(tb)
            bp32 = consts.tile([1, 512], F32)
            nc.sync.dma_start(out=bp32, in_=bproj[:, :])
            bp_bf = consts.tile([1, 512], BF16)
            nc.vector.tensor_copy(bp_bf, bp32)

            def mm_bf16_bank():
                # one PSUM bank viewed as [128, 1024] bf16 for PE transposes
                t = ps_mm.tile([128, 512], F32, tag="mm", name="mmbank")
                return t.bitcast(BF16)

            for pr in range(2):
                # ---- per-patch weights, cast to bf16
                wq_sb, wk_sb, wv_sb = [], [], []
                for ci in range(CCH):
                    rows = slice(ci * 128, (ci + 1) * 128)
                    for lst, src, tag in (
                        (wq_sb, wq[pr, rows, :], f"wq{ci}"),
                        (wk_sb, wkv[pr, rows, 0:512], f"wk{ci}"),
                        (wv_sb, wkv[pr, rows, 512:1024], f"wv{ci}"),
                    ):
                        t32 = stage.tile([128, 512], F32, tag="ws", name="ws")
                        nc.sync.dma_start(out=t32, in_=src)
                        tb = wpool.tile([128, 512], BF16, tag=tag, name=tag)
                        nc.vector.tensor_copy(tb, t32)
                        lst.append(tb)

                # ---- x load, cast to bf16, PE transpose -> xT [c(chunked), n]
                xT = xTp.tile([128, CCH, N], BF16, tag="xT", name="xT")
                for nt in range(NT):
                    xs = stage.tile([128, 512], F32, tag="xs", name="xs")
                    nc.sync.dma_start(out=xs, in_=x[pr, nt * 128 : (nt + 1) * 128, :])
                    xb = stage.tile([128, 512], BF16, tag="xb", name="xb")
                    nc.vector.tensor_copy(xb, xs)
                    pst = mm_bf16_bank()
                    for ci in range(CCH):
                        nc.tensor.transpose(
                            pst[:, ci * 128 : (ci + 1) * 128],
                            xb[:, ci * 128 : (ci + 1) * 128],
                            identbf,
                        )
                    nc.vector.tensor_copy(
                        xT[:, :, nt * 128 : (nt + 1) * 128],
                        pst[:, 0:512].rearrange("p (c n) -> p c n", n=128),
                    )

                # ---- per-di state
                ets = {}  # (di, mt, half) -> exp tile [128, 1024]
                vpad = [None] * NT
                o_sb = [
                    op.tile([128, C], BF16, tag=f"o{nt}", name=f"o{nt}")
                    for nt in range(NT)
                ]

                def emit_qkT(di):
                    qt = qkp.tile([128, N], BF16, tag=f"q{di % 2}", name=f"q{di % 2}")
                    kt = qkp.tile([128, N], BF16, tag=f"k{di % 2}", name=f"k{di % 2}")
                    dcols = slice(di * 128, (di + 1) * 128)
                    for dst, wsb in ((qt, wq_sb), (kt, wk_sb)):
                        for nf in range(2):
                            ncols = slice(nf * 512, (nf + 1) * 512)
                            ps = ps_mm.tile([128, 512], F32, tag="mm", name="mmqk")
                            for ci in range(CCH):
                                nc.tensor.matmul(
                                    ps,
                                    wsb[ci][:, dcols],
                                    xT[:, ci, ncols],
                                    start=(ci == 0),
                                    stop=(ci == CCH - 1),
                                )
                            nc.vector.tensor_copy(dst[:, ncols], ps)
                    return qt, kt

                def emit_v(mt):
                    ps = ps_mm.tile([128, 512], F32, tag="mm", name="mmv")
                    for ci in range(CCH):
                        nc.tensor.matmul(
                            ps,
                            xT[:, ci, mt * 128 : (mt + 1) * 128],
                            wv_sb[ci],
                            start=(ci == 0),
                            stop=(ci == CCH - 1),
                        )
                    vv = vp.tile([128, HEADS * 65], BF16, tag=f"v{mt}", name=f"v{mt}")
                    vr = vv.rearrange("p (h w) -> p h w", w=65)
                    nc.vector.memset(vr[:, :, 64:65], 1.0)
                    nc.vector.tensor_copy(
                        vr[:, :, 0:64], ps.rearrange("p (h w) -> p h w", w=64)
                    )
                    vpad[mt] = vv

                def emit_scores(di, mt, qt, kt):
                    for half in range(2):
                        prow = slice(half * 64, (half + 1) * 64)
                        slab = ps_slab.tile([128, 1024], F32, tag="slab", name="slab")
                        for nf in range(2):
                            nc.tensor.matmul(
                                slab[:, nf * 512 : (nf + 1) * 512],
                                kt[prow, mt * 128 : (mt + 1) * 128],
                                qt[prow, nf * 512 : (nf + 1) * 512],
                                start=True,
                                stop=True,
                            )
                        et = expp.tile(
                            [128, 1024], BF16, tag=f"e{mt}_{half}", name="et"
                        )
                        nc.scalar.activation(
                            et, slab, mybir.ActivationFunctionType.Exp, scale=0.125
                        )
                        ets[(di, mt, half)] = et

                def emit_av(di, nt):
                    av = ps_av.tile([128, 130], F32, tag="av", name="av")
                    for half in range(2):
                        h = 2 * di + half
                        for mt in range(NT):
                            nc.tensor.matmul(
                                av[:, half * 65 : (half + 1) * 65],
                                ets[(di, mt, half)][:, nt * 128 : (nt + 1) * 128],
                                vpad[mt][:, h * 65 : (h + 1) * 65],
                                start=(mt == 0),
                                stop=(mt == NT - 1),
                            )
                    rc = smallp.tile([128, 2], F32, tag="rc", name="rc")
                    nc.vector.reciprocal(rc[:, 0:1], av[:, 64:65])
                    nc.vector.reciprocal(rc[:, 1:2], av[:, 129:130])
                    for half in range(2):
                        h = 2 * di + half
                        nc.vector.tensor_scalar_mul(
                            o_sb[nt][:, h * 64 : (h + 1) * 64],
                            av[:, half * 65 : half * 65 + 64],
                            rc[:, half : half + 1],
                        )

                def emit_proj(nt):
                    pst = mm_bf16_bank()
                    for ci in range(CCH):
                        nc.tensor.transpose(
                            pst[:, ci * 128 : (ci + 1) * 128],
                            o_sb[nt][:, ci * 128 : (ci + 1) * 128],
                            identbf,
                        )
                    oTn = op.tile([128, CCH, 128], BF16, tag="oT", name="oT")
                    nc.vector.tensor_copy(
                        oTn, pst[:, 0:512].rearrange("p (c n) -> p c n", n=128)
                    )
                    zps = ps_mm.tile([128, 512], F32, tag="mm", name="mmz")
                    nc.tensor.matmul(
                        zps, ones_bf[0:1, :], bp_bf[0:1, :], start=True, stop=False
                    )
                    for ci in range(CCH):
                        nc.tensor.matmul(
                            zps,
                            oTn[:, ci, :],
                            wproj_sb[ci],
                            start=False,
                            stop=(ci == CCH - 1),
                        )
                    zsb = zp.tile([128, 512], F32, tag="z", name="z")
                    nc.vector.tensor_copy(zsb, zps)
                    nc.gpsimd.dma_start(
                        out=y[pr, nt * 128 : (nt + 1) * 128, :], in_=zsb
                    )

                # ---- pipelined issue order
                qt, kt = emit_qkT(0)
                for mt in range(NT):
                    emit_scores(0, mt, qt, kt)
                    emit_v(mt)
                for di in range(1, CCH):
                    qt, kt = emit_qkT(di)
                    for mt in range(NT):
                        emit_scores(di, mt, qt, kt)
                        emit_av(di - 1, mt)
                for nt in range(NT):
                    emit_av(CCH - 1, nt)
                    emit_proj(nt)
    return nc


def _get_nc():
    if "nc" not in _CACHE:
        nc = _build_kernel()
        nc.compile()
        _CACHE["nc"] = nc
    return _CACHE["nc"]


def kernel(**inputs) -> np.ndarray:
    from concourse.bass_utils import run_bass_kernel_spmd

    x = np.ascontiguousarray(np.asarray(inputs["x"], dtype=np.float32))
    Wq = np.ascontiguousarray(np.asarray(inputs["Wq"], dtype=np.float32))
    Wkv = np.ascontiguousarray(np.asarray(inputs["Wkv"], dtype=np.float32))
    Wproj = np.ascontiguousarray(np.asarray(inputs["Wproj"], dtype=np.float32))
    bproj = np.ascontiguousarray(
        np.asarray(inputs["bproj"], dtype=np.float32).reshape(1, C)
    )

    nc = _get_nc()
    xr = x.reshape(B * P, N, C)
    in_maps = []
    for core in range(8):
        p0 = (2 * core) % P
        in_maps.append(
            {
                "x": np.ascontiguousarray(xr[2 * core : 2 * core + 2]),
                "wq": np.ascontiguousarray(Wq[p0 : p0 + 2]),
                "wkv": np.ascontiguousarray(Wkv[p0 : p0 + 2]),
                "wproj": Wproj,
                "bproj": bproj,
            }
        )
    res = run_bass_kernel_spmd(nc, in_maps, list(range(8))).results
    out = np.concatenate([r["y"] for r in res], axis=0).reshape(B, P, N, C)
    return out.astype(np.float32)
